# revision 47
# baseline (speedup 1.0000x reference)
"""GCNConv kernel for 8x Trainium2 NeuronCores (Bass/Tile) — v2.

Reference computation:
    h = x @ W + b                  # [N, 256] @ [256, 128] -> [N, 128]
    out[i] = sum_{e: dst[e]=i} val[e] * h[src[e]]

Strategy (per core; SPMD — one program, per-core data):
  - dst nodes sharded 12500/core (output rows); edges partitioned by dst.
  - src rows split into 4 windows of <=32512 rows (int16 gather indices).
  - Processing is window-sweep-outer: for each window w, all 98 dst tiles
    are processed using only h rows of window w; per-tile partial sums
    accumulate in a persistent SBUF f32 accumulator across sweeps.
  - Per (window, tile): edges sorted by src, padded to 128-slot chunks
    (chunk count = max across cores, so one program serves all 8 cores).
    Per chunk: per-edge h rows pulled on-chip with dma_gather (256B rows;
    gathers rotate across the 4 SWDGE queues so all four Q7 core-pairs
    generate descriptors concurrently — 4x the single-queue rate), a
    [128 slot x 128 dst] one-hot-times-val matrix B is generated on-device
    (DVE tensor_scalar: (iota == dlt[p]) * val[p]), and one PE matmul
    accumulates B.T @ msgs into the tile's PSUM.
  - Bias is folded in per tile as a rank-1 matmul: deg[t].T @ bias, where
    deg[i] = sum of edge_vals into dst i (host-computed).
  - Phase 1 (h = x @ W) is pipelined: window w+1's projection runs
    interleaved with sweep w's gathers/matmuls; h windows are separate
    DRAM tensors so the Tile framework serializes only true RAW deps.
"""

import sys

for _p in ("/opt/trn_rl_repo",):
    if _p not in sys.path:
        sys.path.insert(0, _p)

import numpy as np

P = 128
N_CORES = 8
WROWS = 32512          # gather window rows (int16 index limit)
RB = 512               # phase-1 row batch


def _ceil_to(a, m):
    return -(-a // m) * m


class Plan:
    pass


def build_plan(x, edge_src, edge_dst, edge_vals, weight, bias):
    N, IN_F = x.shape
    OUT_F = weight.shape[1]
    assert N % N_CORES == 0
    ndst = N // N_CORES
    ndst_pad = _ceil_to(ndst, P)
    ntile = ndst_pad // P

    NW = -(-N // WROWS)
    wstart = [w * WROWS for w in range(NW)]
    wrows = [min(WROWS, N - s) for s in wstart]
    wrows_pad = [_ceil_to(r, P) for r in wrows]
    hrows_pad = _ceil_to(N, P)          # xT column padding (>= sum windows)
    xt_cols = max(hrows_pad, wstart[-1] + wrows_pad[-1])

    pl = Plan()
    pl.N, pl.IN_F, pl.OUT_F = N, IN_F, OUT_F
    pl.ndst, pl.ndst_pad, pl.ntile = ndst, ndst_pad, ntile
    pl.NW, pl.wstart, pl.wrows, pl.wrows_pad = NW, wstart, wrows, wrows_pad
    pl.kc = IN_F // P
    pl.xt_cols = xt_cols

    # --- dense inputs ---
    xT = np.zeros((pl.kc, P, xt_cols), np.float16)
    xT[:, :, :N] = np.ascontiguousarray(x.astype(np.float16).T).reshape(
        pl.kc, P, N)
    pl.xT = xT
    pl.W = np.ascontiguousarray(
        weight.astype(np.float16).reshape(pl.kc, P, OUT_F).transpose(1, 0, 2))
    pl.bvec = np.ascontiguousarray(bias.astype(np.float16)[None, :])

    # --- per-dst weighted degree (for the bias rank-1 update) ---
    deg_w = np.bincount(edge_dst, weights=edge_vals.astype(np.float64),
                        minlength=N).astype(np.float32)
    degv = np.zeros((N_CORES, 1, ndst_pad), np.float16)
    degv[:, 0, :ndst] = deg_w.reshape(N_CORES, ndst)
    pl.degv = degv

    # --- edge partitioning ---
    src = edge_src.astype(np.int64)
    dst = edge_dst.astype(np.int64)
    val = edge_vals.astype(np.float32)
    core = dst // ndst
    dl = dst % ndst
    tile = dl // P
    w = src // WROWS

    order = np.lexsort((src, tile, w, core))
    src, dl, tile, w, core, val = (src[order], dl[order], tile[order],
                                   w[order], core[order], val[order])

    # edge counts per (core, w, tile)
    cnt = np.zeros((N_CORES, NW, ntile), np.int64)
    np.add.at(cnt, (core, w, tile), 1)
    cmax = cnt.max(axis=0)                       # [NW, ntile]
    nchunk_wt = np.maximum(1, -(-cmax // P))     # >=1 chunk per (w, t)
    pl.nchunk_wt = nchunk_wt
    NCHUNK = int(nchunk_wt.sum())
    TOT = NCHUNK * P
    pl.NCHUNK, pl.TOT = NCHUNK, TOT

    # chunk offsets in processing order (w-major, tile-minor)
    chunk_off = np.zeros((NW, ntile), np.int64)
    off = 0
    wchunk0 = []                                  # first chunk of window w
    for ww in range(NW):
        wchunk0.append(off)
        for t in range(ntile):
            chunk_off[ww, t] = off
            off += int(nchunk_wt[ww, t])
    wchunk0.append(off)
    pl.chunk_off, pl.wchunk0 = chunk_off, wchunk0

    # static chunk program: per chunk -> (w, tile, start, stop)
    chunk_w = np.zeros(NCHUNK, np.int64)
    chunk_t = np.zeros(NCHUNK, np.int64)
    chunk_start = np.zeros(NCHUNK, bool)
    chunk_stop = np.zeros(NCHUNK, bool)
    for ww in range(NW):
        for t in range(ntile):
            c0 = int(chunk_off[ww, t])
            n = int(nchunk_wt[ww, t])
            chunk_w[c0:c0 + n] = ww
            chunk_t[c0:c0 + n] = t
            chunk_start[c0] = True
            chunk_stop[c0 + n - 1] = True
    pl.chunk_w, pl.chunk_t = chunk_w, chunk_t
    pl.chunk_start, pl.chunk_stop = chunk_start, chunk_stop

    # --- per-core slot arrays ---
    # slot position: edges of (core, w, t) go to slots
    # [chunk_off[w,t]*P + k for k in range(cnt)]
    ecount = cnt[core, w, tile]                  # not used; need rank in group
    # rank of each edge within its (core, w, t) group (edges are sorted)
    grp = (core * NW + w) * ntile + tile
    changes = np.ones(len(grp), bool)
    changes[1:] = grp[1:] != grp[:-1]
    gstart = np.nonzero(changes)[0]
    rank = np.arange(len(grp)) - np.repeat(gstart, np.diff(
        np.append(gstart, len(grp))))

    slot = chunk_off[w, tile] * P + rank         # per-edge slot (per-core)

    IDXW = np.zeros((N_CORES, TOT), np.int16)
    dltv = np.full((N_CORES, TOT), -1.0, np.float32)
    valv = np.zeros((N_CORES, TOT), np.float32)
    IDXW[core, slot] = (src - np.array(wstart)[w]).astype(np.int16)
    dltv[core, slot] = (dl % P).astype(np.float32)
    valv[core, slot] = val

    # reshape slot arrays to [P, NCHUNK] (partition = slot % P)
    pl.dlt = np.ascontiguousarray(
        dltv.reshape(N_CORES, NCHUNK, P).transpose(0, 2, 1)).astype(np.float16)
    pl.val = np.ascontiguousarray(
        valv.reshape(N_CORES, NCHUNK, P).transpose(0, 2, 1)).astype(np.float16)

    # gather calls: per window, GMAX-slot pieces; idx tensor wrapped %16
    import os
    GMAX = int(os.environ.get("K_GMAX", "4096"))
    calls = []                                   # (w, slot_off, nslots)
    for ww in range(NW):
        s0, s1 = wchunk0[ww] * P, wchunk0[ww + 1] * P
        for o in range(s0, s1, GMAX):
            calls.append((ww, o, min(GMAX, s1 - o)))
    pl.calls = calls

    IDX = np.zeros((N_CORES, 16, TOT // 16), np.int16)
    for (ww, o, n) in calls:
        IDX[:, :, o // 16:(o + n) // 16] = IDXW[:, o:o + n].reshape(
            N_CORES, n // 16, 16).transpose(0, 2, 1)
    pl.IDX = np.tile(IDX, (1, 8, 1))             # [N_CORES, 128, TOT//16]

    # iota constant [P, P]: row j has value j in each partition
    pl.iota = np.ascontiguousarray(
        np.broadcast_to(np.arange(P, dtype=np.float16)[None, :], (P, P)))
    return pl


# ---------------------------------------------------------------------------
# Device program
# ---------------------------------------------------------------------------

def build_bass(pl):
    import os
    import concourse.bass as bass
    import concourse.mybir as mybir
    import concourse.tile as tile
    from concourse import bacc

    f16 = mybir.dt.float16
    f32 = mybir.dt.float32
    i16 = mybir.dt.int16

    NSWQ = int(os.environ.get("K_NSWQ", "4"))
    P1SPREAD = int(os.environ.get("K_P1SPREAD", "5"))
    BGB = int(os.environ.get("K_BGB", "16"))
    nc = bacc.Bacc("TRN2", target_bir_lowering=False, debug=False,
                   num_swdge_queues=NSWQ)

    OF = pl.OUT_F
    xT_d = nc.dram_tensor("xt", [pl.kc, P, pl.xt_cols], f16,
                          kind="ExternalInput")
    W_d = nc.dram_tensor("w", [P, pl.kc, OF], f16, kind="ExternalInput")
    b_d = nc.dram_tensor("bvec", [1, OF], f16, kind="ExternalInput")
    deg_d = nc.dram_tensor("degv", [1, pl.ndst_pad], f16,
                           kind="ExternalInput")
    iota_d = nc.dram_tensor("iota", [P, P], f16, kind="ExternalInput")
    idx_d = nc.dram_tensor("idx", [P, pl.TOT // 16], i16,
                           kind="ExternalInput")
    dlt_d = nc.dram_tensor("dlt", [P, pl.NCHUNK], f16, kind="ExternalInput")
    val32_d = nc.dram_tensor("val32", [P, pl.NCHUNK], f32,
                             kind="ExternalInput")
    out_d = nc.dram_tensor("out", [pl.ndst_pad, OF], f32,
                           kind="ExternalOutput")
    h_ds = [nc.dram_tensor(f"hbuf{w}", [pl.wrows_pad[w], OF], f16)
            for w in range(pl.NW)]

    with tile.TileContext(nc) as tc:
        with (
            tc.tile_pool(name="pconst", bufs=1) as pconst,
            tc.tile_pool(name="pacc", bufs=1) as pacc,
            tc.tile_pool(name="pxt", bufs=4) as pxt,
            tc.tile_pool(name="phs", bufs=4) as phs,
            tc.tile_pool(name="pp1", bufs=2, space="PSUM") as pp1,
            tc.tile_pool(name="pidx", bufs=2) as pidx,
            tc.tile_pool(name="pmsg", bufs=int(os.environ.get(
                "K_PMSG", "10"))) as pmsg,
            tc.tile_pool(name="pB", bufs=4) as pB,
            tc.tile_pool(name="pout", bufs=3) as pout,
            tc.tile_pool(name="psL1", bufs=6, space="PSUM") as psL1,
        ):
            W_sb = pconst.tile([P, pl.kc, OF], f16)
            nc.sync.dma_start(W_sb[:], W_d[:])
            b_sb = pconst.tile([1, OF], f16)
            nc.sync.dma_start(b_sb[:], b_d[:])
            deg_sb = pconst.tile([1, pl.ndst_pad], f16)
            nc.sync.dma_start(deg_sb[:], deg_d[:])
            iota_sb = pconst.tile([P, P], f16)
            nc.sync.dma_start(iota_sb[:], iota_d[:])
            dlt_sb = pconst.tile([P, pl.NCHUNK], f16)
            nc.sync.dma_start(dlt_sb[:], dlt_d[:])
            val32_sb = pconst.tile([P, pl.NCHUNK], f32)
            nc.sync.dma_start(val32_sb[:], val32_d[:])
            acc = pacc.tile([P, pl.ntile, OF], f16)

            # ---------------- phase-1 batch generator ----------------
            def phase1_batches(w):
                nrows_w = pl.wrows_pad[w]
                for bi, r0 in enumerate(range(0, nrows_w, RB)):
                    nrows = min(RB, nrows_w - r0)
                    abs0 = pl.wstart[w] + r0
                    nch = nrows // P
                    xt = pxt.tile([P, pl.kc, RB], f16, tag="xt")
                    dma_eng = nc.scalar if (w == 0 and bi % 2) else nc.sync
                    dma_eng.dma_start(
                        xt[:, :, :nrows],
                        xT_d[:, :, abs0:abs0 + nrows].rearrange(
                            "k p c -> p k c"),
                    )
                    ps = pp1.tile([P, RB], f32, tag="pj")
                    for rc in range(nch):
                        for k in range(pl.kc):
                            nc.tensor.matmul(
                                ps[:, rc * P:(rc + 1) * P],
                                lhsT=xt[:, k, rc * P:(rc + 1) * P],
                                rhs=W_sb[:, k, :],
                                start=(k == 0),
                                stop=(k == pl.kc - 1),
                            )
                    hs = phs.tile([P, RB], f16, tag="hs")
                    nc.scalar.activation(
                        hs[:, :nrows], ps[:, :nrows],
                        mybir.ActivationFunctionType.Copy)
                    nc.sync.dma_start(
                        h_ds[w][r0:r0 + nrows, :].rearrange(
                            "(c p) f -> p c f", p=P),
                        hs[:, :nrows].rearrange("p (c f) -> p c f", f=OF),
                    )
                    yield

            # ---------------- main pipeline ----------------
            gens = [phase1_batches(w) for w in range(pl.NW)]

            def drain(g, k=None):
                i = 0
                for _ in g:
                    i += 1
                    if k is not None and i >= k:
                        return

            drain(gens[0])                       # h window 0 fully projected

            gq = 0                               # gather queue rotation
            psum = {}                            # tile -> psum tile

            # half-sweep idx blocks: each sweep's calls split in two
            # contiguous groups; the NEXT half's idx block is DMA'd when a
            # half begins, so the (bufs=2) pool double-buffers.
            halves = []                          # (slot_lo, slot_hi)
            call_half = []                       # call idx -> half idx
            wcalls = {}
            for ci, (w, o, n) in enumerate(pl.calls):
                wcalls.setdefault(w, []).append(ci)
            for w in range(pl.NW):
                cs = wcalls[w]
                mid = (len(cs) + 1) // 2
                for grp in (cs[:mid], cs[mid:]):
                    if not grp:
                        continue
                    lo = pl.calls[grp[0]][1]
                    hi = pl.calls[grp[-1]][1] + pl.calls[grp[-1]][2]
                    hidx = len(halves)
                    halves.append((lo, hi))
                    for ci in grp:
                        while len(call_half) <= ci:
                            call_half.append(hidx)
            half_tiles = {}

            def preload_half(j):
                lo, hi = halves[j]
                sxt = pidx.tile([P, (hi - lo) // 16], i16, tag="idx",
                                name=f"idx_h{j}")
                nc.sync.dma_start(sxt[:], idx_d[:, lo // 16:hi // 16])
                half_tiles[j] = (sxt, lo)

            preload_half(0)
            for ci, (w, o, n) in enumerate(pl.calls):
                j = call_half[ci]
                if ci == 0 or call_half[ci - 1] != j:
                    if j + 1 < len(halves):
                        preload_half(j + 1)
                # interleave next window's projection
                if w + 1 < pl.NW:
                    drain(gens[w + 1], P1SPREAD)
                sxt, s0 = half_tiles[call_half[ci]]
                mt = pmsg.tile([P, n // P, OF], f16, tag="msg")
                nc.gpsimd.dma_gather(
                    out_ap=mt[:],
                    in_ap=h_ds[w][:, :],
                    idxs_ap=sxt[:, (o - s0) // 16:(o - s0 + n) // 16],
                    num_idxs=n,
                    num_idxs_reg=n,
                    elem_size=OF,
                    single_packet=os.environ.get("K_SP", "0") == "1",
                    queue_num=gq % NSWQ,
                )
                gq += 1
                c0, c1 = o // P, (o + n) // P
                Bt = None
                for c in range(c0, c1):
                    t = int(pl.chunk_t[c])
                    if pl.chunk_start[c]:
                        psum[t] = psL1.tile([P, OF], f32, tag="l1",
                                            name=f"ps_{w}_{t}")
                    if (c - c0) % BGB == 0:
                        # batched B generation: k chunks per DVE op pair
                        b0 = c
                        k = min(BGB, c1 - c)
                        Bt = pB.tile([P, BGB, P], f16, tag="B")
                        ia = iota_sb[:]
                        da = dlt_sb[:, b0:b0 + k]
                        va = val32_sb[:, b0:b0 + k]
                        iota_b = bass.AP(
                            ia.tensor, ia.offset,
                            [list(ia.ap[0]), [0, k], list(ia.ap[1])])
                        dlt_b = bass.AP(
                            da.tensor, da.offset,
                            [list(da.ap[0]), list(da.ap[1]), [0, P]])
                        val_b = bass.AP(
                            va.tensor, va.offset,
                            [list(va.ap[0]), list(va.ap[1]), [0, P]])
                        nc.vector.tensor_tensor(
                            Bt[:, :k, :], iota_b, dlt_b,
                            mybir.AluOpType.is_equal)
                        if ((c - c0) // BGB) % 4 == 0:
                            nc.vector.tensor_tensor(
                                Bt[:, :k, :], Bt[:, :k, :], val_b,
                                mybir.AluOpType.mult)
                        else:
                            # spread the val-scale pass onto the idle ACT
                            # engine (per-chunk per-partition scale)
                            for cc in range(b0, b0 + k):
                                nc.scalar.activation(
                                    Bt[:, cc - b0, :], Bt[:, cc - b0, :],
                                    mybir.ActivationFunctionType.Copy,
                                    scale=val32_sb[:, cc:cc + 1])
                    last = bool(pl.chunk_stop[c])
                    fin = last and w == pl.NW - 1
                    nc.tensor.matmul(
                        psum[t][:],
                        lhsT=Bt[:, (c - b0), :],
                        rhs=mt[:, c - c0, :],
                        start=bool(pl.chunk_start[c]),
                        stop=last and not fin,
                    )
                    if fin:                      # bias rank-1, then evict
                        nc.tensor.matmul(
                            psum[t][:],
                            lhsT=deg_sb[:, t * P:(t + 1) * P],
                            rhs=b_sb[:],
                            start=False,
                            stop=True,
                        )
                    if last:
                        if w == 0:
                            nc.scalar.activation(
                                acc[:, t, :], psum[t][:],
                                mybir.ActivationFunctionType.Copy)
                        elif not fin:
                            nc.vector.tensor_tensor(
                                acc[:, t, :], acc[:, t, :], psum[t][:],
                                mybir.AluOpType.add)
                        else:
                            ot = pout.tile([P, OF], f32, tag="out")
                            nc.vector.tensor_tensor(
                                ot[:], acc[:, t, :], psum[t][:],
                                mybir.AluOpType.add)
                            nc.scalar.dma_start(out_d[t * P:(t + 1) * P, :],
                                                ot[:])
                        del psum[t]

    nc.compile()
    return nc


# ---------------------------------------------------------------------------
# Entry point
# ---------------------------------------------------------------------------

def kernel(x, edge_src, edge_dst, edge_vals, weight, bias,
           _want_trace=False, _n_cores=None):
    x = np.asarray(x)
    edge_src = np.asarray(edge_src)
    edge_dst = np.asarray(edge_dst)
    edge_vals = np.asarray(edge_vals)
    weight = np.asarray(weight)
    bias = np.asarray(bias)

    pl = build_plan(x, edge_src, edge_dst, edge_vals, weight, bias)
    nc = build_bass(pl)

    from concourse.bass_utils import run_bass_kernel_spmd

    ncores = N_CORES if _n_cores is None else _n_cores
    in_maps = []
    for ci in range(ncores):
        in_maps.append({
            "xt": pl.xT,
            "w": pl.W,
            "bvec": pl.bvec,
            "degv": np.ascontiguousarray(pl.degv[ci]),
            "iota": pl.iota,
            "idx": np.ascontiguousarray(pl.IDX[ci]),
            "dlt": np.ascontiguousarray(pl.dlt[ci]),
            "val32": np.ascontiguousarray(pl.val[ci]).astype(np.float32),
        })
    res = run_bass_kernel_spmd(nc, in_maps, core_ids=list(range(ncores)),
                               trace=_want_trace)
    outs = [res.results[ci]["out"][:pl.ndst, :] for ci in range(ncores)]
    if ncores < N_CORES:
        outs += [np.zeros((pl.ndst, pl.OUT_F), np.float32)] * (
            N_CORES - ncores)
    full = np.concatenate(outs, axis=0).astype(np.float32)
    if _want_trace:
        kernel._last_results = res
    return full


# revision 48
# speedup vs baseline: 1.2060x; 1.2060x over previous
"""GCNConv kernel for 8x Trainium2 NeuronCores (Bass/Tile) — v2.

Reference computation:
    h = x @ W + b                  # [N, 256] @ [256, 128] -> [N, 128]
    out[i] = sum_{e: dst[e]=i} val[e] * h[src[e]]

Strategy (per core; SPMD — one program, per-core data):
  - dst nodes sharded 12500/core (output rows); edges partitioned by dst.
  - src rows split into 4 windows of <=32512 rows (int16 gather indices).
  - Processing is window-sweep-outer: for each window w, all 98 dst tiles
    are processed using only h rows of window w; per-tile partial sums
    accumulate in a persistent SBUF f32 accumulator across sweeps.
  - Per (window, tile): edges sorted by src, padded to 128-slot chunks
    (chunk count = max across cores, so one program serves all 8 cores).
    Per chunk: per-edge h rows pulled on-chip with dma_gather (256B rows;
    gathers rotate across the 4 SWDGE queues so all four Q7 core-pairs
    generate descriptors concurrently — 4x the single-queue rate), a
    [128 slot x 128 dst] one-hot-times-val matrix B is generated on-device
    (DVE tensor_scalar: (iota == dlt[p]) * val[p]), and one PE matmul
    accumulates B.T @ msgs into the tile's PSUM.
  - Bias is folded in per tile as a rank-1 matmul: deg[t].T @ bias, where
    deg[i] = sum of edge_vals into dst i (host-computed).
  - Phase 1 (h = x @ W) is pipelined: window w+1's projection runs
    interleaved with sweep w's gathers/matmuls; h windows are separate
    DRAM tensors so the Tile framework serializes only true RAW deps.
"""

import sys

for _p in ("/opt/trn_rl_repo",):
    if _p not in sys.path:
        sys.path.insert(0, _p)

import numpy as np

P = 128
N_CORES = 8
WROWS = 32512          # gather window rows (int16 index limit)
RB = 512               # phase-1 row batch


def _ceil_to(a, m):
    return -(-a // m) * m


class Plan:
    pass


def build_plan(x, edge_src, edge_dst, edge_vals, weight, bias):
    N, IN_F = x.shape
    OUT_F = weight.shape[1]
    assert N % N_CORES == 0
    ndst = N // N_CORES
    ndst_pad = _ceil_to(ndst, P)
    ntile = ndst_pad // P

    NW = -(-N // WROWS)
    wstart = [w * WROWS for w in range(NW)]
    wrows = [min(WROWS, N - s) for s in wstart]
    wrows_pad = [_ceil_to(r, P) for r in wrows]
    hrows_pad = _ceil_to(N, P)          # xT column padding (>= sum windows)
    xt_cols = max(hrows_pad, wstart[-1] + wrows_pad[-1])

    pl = Plan()
    pl.N, pl.IN_F, pl.OUT_F = N, IN_F, OUT_F
    pl.ndst, pl.ndst_pad, pl.ntile = ndst, ndst_pad, ntile
    pl.NW, pl.wstart, pl.wrows, pl.wrows_pad = NW, wstart, wrows, wrows_pad
    pl.kc = IN_F // P
    pl.xt_cols = xt_cols

    # --- dense inputs ---
    xT = np.zeros((pl.kc, P, xt_cols), np.float16)
    xT[:, :, :N] = np.ascontiguousarray(x.astype(np.float16).T).reshape(
        pl.kc, P, N)
    pl.xT = xT
    pl.W = np.ascontiguousarray(
        weight.astype(np.float16).reshape(pl.kc, P, OUT_F).transpose(1, 0, 2))
    pl.bvec = np.ascontiguousarray(bias.astype(np.float16)[None, :])

    # --- per-dst weighted degree (for the bias rank-1 update) ---
    deg_w = np.bincount(edge_dst, weights=edge_vals.astype(np.float64),
                        minlength=N).astype(np.float32)
    degv = np.zeros((N_CORES, 1, ndst_pad), np.float16)
    degv[:, 0, :ndst] = deg_w.reshape(N_CORES, ndst)
    pl.degv = degv

    # --- edge partitioning ---
    src = edge_src.astype(np.int64)
    dst = edge_dst.astype(np.int64)
    val = edge_vals.astype(np.float32)
    core = dst // ndst
    dl = dst % ndst
    tile = dl // P
    w = src // WROWS

    order = np.lexsort((src, tile, w, core))
    src, dl, tile, w, core, val = (src[order], dl[order], tile[order],
                                   w[order], core[order], val[order])

    # edge counts per (core, w, tile)
    cnt = np.zeros((N_CORES, NW, ntile), np.int64)
    np.add.at(cnt, (core, w, tile), 1)
    cmax = cnt.max(axis=0)                       # [NW, ntile]
    nchunk_wt = np.maximum(1, -(-cmax // P))     # >=1 chunk per (w, t)
    pl.nchunk_wt = nchunk_wt
    NCHUNK = int(nchunk_wt.sum())
    TOT = NCHUNK * P
    pl.NCHUNK, pl.TOT = NCHUNK, TOT

    # chunk offsets in processing order (w-major, tile-minor)
    chunk_off = np.zeros((NW, ntile), np.int64)
    off = 0
    wchunk0 = []                                  # first chunk of window w
    for ww in range(NW):
        wchunk0.append(off)
        for t in range(ntile):
            chunk_off[ww, t] = off
            off += int(nchunk_wt[ww, t])
    wchunk0.append(off)
    pl.chunk_off, pl.wchunk0 = chunk_off, wchunk0

    # static chunk program: per chunk -> (w, tile, start, stop)
    chunk_w = np.zeros(NCHUNK, np.int64)
    chunk_t = np.zeros(NCHUNK, np.int64)
    chunk_start = np.zeros(NCHUNK, bool)
    chunk_stop = np.zeros(NCHUNK, bool)
    for ww in range(NW):
        for t in range(ntile):
            c0 = int(chunk_off[ww, t])
            n = int(nchunk_wt[ww, t])
            chunk_w[c0:c0 + n] = ww
            chunk_t[c0:c0 + n] = t
            chunk_start[c0] = True
            chunk_stop[c0 + n - 1] = True
    pl.chunk_w, pl.chunk_t = chunk_w, chunk_t
    pl.chunk_start, pl.chunk_stop = chunk_start, chunk_stop

    # --- per-core slot arrays ---
    # slot position: edges of (core, w, t) go to slots
    # [chunk_off[w,t]*P + k for k in range(cnt)]
    ecount = cnt[core, w, tile]                  # not used; need rank in group
    # rank of each edge within its (core, w, t) group (edges are sorted)
    grp = (core * NW + w) * ntile + tile
    changes = np.ones(len(grp), bool)
    changes[1:] = grp[1:] != grp[:-1]
    gstart = np.nonzero(changes)[0]
    rank = np.arange(len(grp)) - np.repeat(gstart, np.diff(
        np.append(gstart, len(grp))))

    slot = chunk_off[w, tile] * P + rank         # per-edge slot (per-core)

    IDXW = np.zeros((N_CORES, TOT), np.int16)
    dltv = np.full((N_CORES, TOT), -1.0, np.float32)
    valv = np.zeros((N_CORES, TOT), np.float32)
    IDXW[core, slot] = (src - np.array(wstart)[w]).astype(np.int16)
    dltv[core, slot] = (dl % P).astype(np.float32)
    valv[core, slot] = val

    # reshape slot arrays to [P, NCHUNK] (partition = slot % P)
    pl.dlt = np.ascontiguousarray(
        dltv.reshape(N_CORES, NCHUNK, P).transpose(0, 2, 1)).astype(np.float16)
    pl.val = np.ascontiguousarray(
        valv.reshape(N_CORES, NCHUNK, P).transpose(0, 2, 1)).astype(np.float16)

    # gather calls: per window, GMAX-slot pieces; idx tensor wrapped %16
    import os
    GMAX = int(os.environ.get("K_GMAX", "4096"))
    calls = []                                   # (w, slot_off, nslots)
    for ww in range(NW):
        s0, s1 = wchunk0[ww] * P, wchunk0[ww + 1] * P
        for o in range(s0, s1, GMAX):
            calls.append((ww, o, min(GMAX, s1 - o)))
    pl.calls = calls

    IDX = np.zeros((N_CORES, 16, TOT // 16), np.int16)
    for (ww, o, n) in calls:
        IDX[:, :, o // 16:(o + n) // 16] = IDXW[:, o:o + n].reshape(
            N_CORES, n // 16, 16).transpose(0, 2, 1)
    pl.IDX = np.tile(IDX, (1, 8, 1))             # [N_CORES, 128, TOT//16]

    # iota constant [P, P]: row j has value j in each partition
    pl.iota = np.ascontiguousarray(
        np.broadcast_to(np.arange(P, dtype=np.float16)[None, :], (P, P)))
    return pl


# ---------------------------------------------------------------------------
# Device program
# ---------------------------------------------------------------------------

def build_bass(pl):
    import os
    import concourse.bass as bass
    import concourse.mybir as mybir
    import concourse.tile as tile
    from concourse import bacc

    f16 = mybir.dt.float16
    f32 = mybir.dt.float32
    i16 = mybir.dt.int16

    NSWQ = int(os.environ.get("K_NSWQ", "4"))
    P1SPREAD = int(os.environ.get("K_P1SPREAD", "5"))
    BGB = int(os.environ.get("K_BGB", "16"))
    nc = bacc.Bacc("TRN2", target_bir_lowering=False, debug=False,
                   num_swdge_queues=NSWQ)

    OF = pl.OUT_F
    xT_d = nc.dram_tensor("xt", [pl.kc, P, pl.xt_cols], f16,
                          kind="ExternalInput")
    W_d = nc.dram_tensor("w", [P, pl.kc, OF], f16, kind="ExternalInput")
    b_d = nc.dram_tensor("bvec", [1, OF], f16, kind="ExternalInput")
    deg_d = nc.dram_tensor("degv", [1, pl.ndst_pad], f16,
                           kind="ExternalInput")
    iota_d = nc.dram_tensor("iota", [P, P], f16, kind="ExternalInput")
    idx_d = nc.dram_tensor("idx", [P, pl.TOT // 16], i16,
                           kind="ExternalInput")
    dlt_d = nc.dram_tensor("dlt", [P, pl.NCHUNK], f16, kind="ExternalInput")
    val32_d = nc.dram_tensor("val32", [P, pl.NCHUNK], f32,
                             kind="ExternalInput")
    out_d = nc.dram_tensor("out", [pl.ndst_pad, OF], f32,
                           kind="ExternalOutput")
    h_ds = [nc.dram_tensor(f"hbuf{w}", [pl.wrows_pad[w], OF], f16)
            for w in range(pl.NW)]

    with tile.TileContext(nc) as tc:
        with (
            tc.tile_pool(name="pconst", bufs=1) as pconst,
            tc.tile_pool(name="pacc", bufs=1) as pacc,
            tc.tile_pool(name="pxt", bufs=4) as pxt,
            tc.tile_pool(name="phs", bufs=4) as phs,
            tc.tile_pool(name="pp1", bufs=2, space="PSUM") as pp1,
            tc.tile_pool(name="pidx", bufs=2) as pidx,
            tc.tile_pool(name="pmsg", bufs=int(os.environ.get(
                "K_PMSG", "10"))) as pmsg,
            tc.tile_pool(name="pB", bufs=4) as pB,
            tc.tile_pool(name="pout", bufs=3) as pout,
            tc.tile_pool(name="psL1", bufs=6, space="PSUM") as psL1,
        ):
            W_sb = pconst.tile([P, pl.kc, OF], f16)
            nc.sync.dma_start(W_sb[:], W_d[:])
            b_sb = pconst.tile([1, OF], f16)
            nc.sync.dma_start(b_sb[:], b_d[:])
            deg_sb = pconst.tile([1, pl.ndst_pad], f16)
            nc.sync.dma_start(deg_sb[:], deg_d[:])
            iota_sb = pconst.tile([P, P], f16)
            nc.sync.dma_start(iota_sb[:], iota_d[:])
            dlt_sb = pconst.tile([P, pl.NCHUNK], f16)
            nc.sync.dma_start(dlt_sb[:], dlt_d[:])
            val32_sb = pconst.tile([P, pl.NCHUNK], f32)
            nc.sync.dma_start(val32_sb[:], val32_d[:])
            acc = pacc.tile([P, pl.ntile, OF], f16)

            # ---------------- phase-1 batch generator ----------------
            def phase1_batches(w):
                nrows_w = pl.wrows_pad[w]
                for bi, r0 in enumerate(range(0, nrows_w, RB)):
                    nrows = min(RB, nrows_w - r0)
                    abs0 = pl.wstart[w] + r0
                    nch = nrows // P
                    xt = pxt.tile([P, pl.kc, RB], f16, tag="xt")
                    dma_eng = nc.scalar if (w == 0 and bi % 2) else nc.sync
                    dma_eng.dma_start(
                        xt[:, :, :nrows],
                        xT_d[:, :, abs0:abs0 + nrows].rearrange(
                            "k p c -> p k c"),
                    )
                    ps = pp1.tile([P, RB], f32, tag="pj")
                    for rc in range(nch):
                        for k in range(pl.kc):
                            nc.tensor.matmul(
                                ps[:, rc * P:(rc + 1) * P],
                                lhsT=xt[:, k, rc * P:(rc + 1) * P],
                                rhs=W_sb[:, k, :],
                                start=(k == 0),
                                stop=(k == pl.kc - 1),
                            )
                    hs = phs.tile([P, RB], f16, tag="hs")
                    nc.vector.tensor_copy(hs[:, :nrows], ps[:, :nrows])
                    nc.sync.dma_start(
                        h_ds[w][r0:r0 + nrows, :].rearrange(
                            "(c p) f -> p c f", p=P),
                        hs[:, :nrows].rearrange("p (c f) -> p c f", f=OF),
                    )
                    yield

            # ---------------- main pipeline ----------------
            gens = [phase1_batches(w) for w in range(pl.NW)]

            def drain(g, k=None):
                i = 0
                for _ in g:
                    i += 1
                    if k is not None and i >= k:
                        return

            drain(gens[0])                       # h window 0 fully projected

            gq = 0                               # gather queue rotation
            psum = {}                            # tile -> psum tile

            # half-sweep idx blocks: each sweep's calls split in two
            # contiguous groups; the NEXT half's idx block is DMA'd when a
            # half begins, so the (bufs=2) pool double-buffers.
            halves = []                          # (slot_lo, slot_hi)
            call_half = []                       # call idx -> half idx
            wcalls = {}
            for ci, (w, o, n) in enumerate(pl.calls):
                wcalls.setdefault(w, []).append(ci)
            for w in range(pl.NW):
                cs = wcalls[w]
                mid = (len(cs) + 1) // 2
                for grp in (cs[:mid], cs[mid:]):
                    if not grp:
                        continue
                    lo = pl.calls[grp[0]][1]
                    hi = pl.calls[grp[-1]][1] + pl.calls[grp[-1]][2]
                    hidx = len(halves)
                    halves.append((lo, hi))
                    for ci in grp:
                        while len(call_half) <= ci:
                            call_half.append(hidx)
            half_tiles = {}

            def preload_half(j):
                lo, hi = halves[j]
                sxt = pidx.tile([P, (hi - lo) // 16], i16, tag="idx",
                                name=f"idx_h{j}")
                nc.sync.dma_start(sxt[:], idx_d[:, lo // 16:hi // 16])
                half_tiles[j] = (sxt, lo)

            preload_half(0)
            for ci, (w, o, n) in enumerate(pl.calls):
                j = call_half[ci]
                if ci == 0 or call_half[ci - 1] != j:
                    if j + 1 < len(halves):
                        preload_half(j + 1)
                # interleave next window's projection
                if w + 1 < pl.NW:
                    drain(gens[w + 1], P1SPREAD)
                sxt, s0 = half_tiles[call_half[ci]]
                mt = pmsg.tile([P, n // P, OF], f16, tag="msg")
                nc.gpsimd.dma_gather(
                    out_ap=mt[:],
                    in_ap=h_ds[w][:, :],
                    idxs_ap=sxt[:, (o - s0) // 16:(o - s0 + n) // 16],
                    num_idxs=n,
                    num_idxs_reg=n,
                    elem_size=OF,
                    single_packet=os.environ.get("K_SP", "0") == "1",
                    queue_num=gq % NSWQ,
                )
                gq += 1
                c0, c1 = o // P, (o + n) // P
                Bt = None
                for c in range(c0, c1):
                    t = int(pl.chunk_t[c])
                    if pl.chunk_start[c]:
                        psum[t] = psL1.tile([P, OF], f32, tag="l1",
                                            name=f"ps_{w}_{t}")
                    if (c - c0) % BGB == 0:
                        # batched B generation: k chunks per DVE op pair
                        b0 = c
                        k = min(BGB, c1 - c)
                        Bt = pB.tile([P, BGB, P], f16, tag="B")
                        ia = iota_sb[:]
                        da = dlt_sb[:, b0:b0 + k]
                        va = val32_sb[:, b0:b0 + k]
                        iota_b = bass.AP(
                            ia.tensor, ia.offset,
                            [list(ia.ap[0]), [0, k], list(ia.ap[1])])
                        dlt_b = bass.AP(
                            da.tensor, da.offset,
                            [list(da.ap[0]), list(da.ap[1]), [0, P]])
                        val_b = bass.AP(
                            va.tensor, va.offset,
                            [list(va.ap[0]), list(va.ap[1]), [0, P]])
                        nc.vector.tensor_tensor(
                            Bt[:, :k, :], iota_b, dlt_b,
                            mybir.AluOpType.is_equal)
                        if ((c - c0) // BGB) % 4 == 0:
                            nc.vector.tensor_tensor(
                                Bt[:, :k, :], Bt[:, :k, :], val_b,
                                mybir.AluOpType.mult)
                        else:
                            # spread the val-scale pass onto the idle ACT
                            # engine (per-chunk per-partition scale)
                            for cc in range(b0, b0 + k):
                                nc.scalar.activation(
                                    Bt[:, cc - b0, :], Bt[:, cc - b0, :],
                                    mybir.ActivationFunctionType.Copy,
                                    scale=val32_sb[:, cc:cc + 1])
                    last = bool(pl.chunk_stop[c])
                    fin = last and w == pl.NW - 1
                    nc.tensor.matmul(
                        psum[t][:],
                        lhsT=Bt[:, (c - b0), :],
                        rhs=mt[:, c - c0, :],
                        start=bool(pl.chunk_start[c]),
                        stop=last and not fin,
                    )
                    if fin:                      # bias rank-1, then evict
                        nc.tensor.matmul(
                            psum[t][:],
                            lhsT=deg_sb[:, t * P:(t + 1) * P],
                            rhs=b_sb[:],
                            start=False,
                            stop=True,
                        )
                    if last:
                        if w == 0:
                            nc.scalar.activation(
                                acc[:, t, :], psum[t][:],
                                mybir.ActivationFunctionType.Copy)
                        elif not fin:
                            nc.vector.tensor_tensor(
                                acc[:, t, :], acc[:, t, :], psum[t][:],
                                mybir.AluOpType.add)
                        else:
                            ot = pout.tile([P, OF], f32, tag="out")
                            nc.vector.tensor_tensor(
                                ot[:], acc[:, t, :], psum[t][:],
                                mybir.AluOpType.add)
                            nc.scalar.dma_start(out_d[t * P:(t + 1) * P, :],
                                                ot[:])
                        del psum[t]

    nc.compile()
    return nc


# ---------------------------------------------------------------------------
# Entry point
# ---------------------------------------------------------------------------

def kernel(x, edge_src, edge_dst, edge_vals, weight, bias,
           _want_trace=False, _n_cores=None):
    x = np.asarray(x)
    edge_src = np.asarray(edge_src)
    edge_dst = np.asarray(edge_dst)
    edge_vals = np.asarray(edge_vals)
    weight = np.asarray(weight)
    bias = np.asarray(bias)

    pl = build_plan(x, edge_src, edge_dst, edge_vals, weight, bias)
    nc = build_bass(pl)

    from concourse.bass_utils import run_bass_kernel_spmd

    ncores = N_CORES if _n_cores is None else _n_cores
    in_maps = []
    for ci in range(ncores):
        in_maps.append({
            "xt": pl.xT,
            "w": pl.W,
            "bvec": pl.bvec,
            "degv": np.ascontiguousarray(pl.degv[ci]),
            "iota": pl.iota,
            "idx": np.ascontiguousarray(pl.IDX[ci]),
            "dlt": np.ascontiguousarray(pl.dlt[ci]),
            "val32": np.ascontiguousarray(pl.val[ci]).astype(np.float32),
        })
    res = run_bass_kernel_spmd(nc, in_maps, core_ids=list(range(ncores)),
                               trace=_want_trace)
    outs = [res.results[ci]["out"][:pl.ndst, :] for ci in range(ncores)]
    if ncores < N_CORES:
        outs += [np.zeros((pl.ndst, pl.OUT_F), np.float32)] * (
            N_CORES - ncores)
    full = np.concatenate(outs, axis=0).astype(np.float32)
    if _want_trace:
        kernel._last_results = res
    return full
